# revision 15
# baseline (speedup 1.0000x reference)
"""AttentionPool Trainium2 kernel.

Computes, for x [B, N, D], mask [B, N], q [D]:
    logits = einsum('bnd,d->bn', x, q);  logits[~mask] = -inf
    w = softmax(logits, axis=-1)
    out = einsum('bn,bnd->bd', w, x)

Sharding: data-parallel over B across 8 NeuronCores (4 rows per core).

Position enumeration (per row): n = t8*1024 + p*8 + s, with p = SBUF
partition, s in [0,8), t8 in [0,8). Each partition reads 8 consecutive
positions = 8 KiB contiguous DRAM per (p, t8) -> one fat DMA descriptor.
A "tile" is (t8, s): 128 positions, one per partition; col = t8*8 + s.

Per-core device program (per batch row):
  - DMA the row into SBUF in 8 chunks (one per t8), f32.
  - ScalarE casts each chunk to bf16 (for pass 2).
  - Logits on DVE via a custom scan op (registered in-process; ships its own
    uop tables in the NEFF): one op per chunk computes the running prefix of
    x*q over 2048 elements; a stride-0 output AP keeps only each 256-element
    segment end -> 8 segment sums per op at ~1.09 cycles/element.
    Tile dot products = adjacent-difference of segment ends (one DVE
    subtract per row, with a zero column between chunk groups), + mask bias.
  - Row max: DVE free-dim reduce_max + GPSIMD partition_all_reduce(max).
  - w = exp(logits - max) on ScalarE (bf16 out), accum_out -> per-partition
    exp-sums (f32); Z summed on host.
  - Pass 2 on TensorE in bf16: out[1, d] += w[:, col].T @ x_tile[:, d]
    (stationary = the w column, M=1 -> LDWEIGHTS ~1 cycle), PSUM accumulate.
  - Host divides by Z.
"""

import numpy as np

B, N, D = 32, 8192, 256
N_CORES = 8
B_LOC = B // N_CORES  # 4
P = 128
S = 8               # consecutive positions per partition (8 KiB descriptors)
T8 = N // (P * S)   # 8 chunk groups per row
T = N // P          # 64 tiles (columns) per row
NCHUNK = T8         # one DMA chunk per t8 group
GK = 9              # ends layout: 1 zero col + 8 segment ends per chunk

USE_BF16_PASS2 = True

_cache = {}

_SCAN_OP_NAME = "ATTNPOOL_MUL_SCAN"


def _register_scan_op():
    """Register a custom DVE op computing scan(add, Src0*Src1) in-process.

    The stock TENSOR_TENSOR_REDUCE / TENSOR_TENSOR_SCAN opcodes crash this
    terminal's ucode; custom-DVE ops ship their own uop tables inside the
    NEFF, so they are self-contained.
    """
    from concourse import dve_ops
    from concourse.dve_spec import AluOp, Spec, Src0, Src1, scan, lower, _has_src1
    from concourse.dve_uop import DveOpSpec

    for op in dve_ops.OPS:
        if op.name == _SCAN_OP_NAME:
            return op
    spec = Spec(
        body=scan(AluOp.ADD, Src0 * Src1),
        reference=lambda in0, in1, c0, c1, c2: np.cumsum(
            in0.astype(np.float32) * in1, axis=1, dtype=np.float32
        ),
    )
    row = dve_ops._CUSTOM_DVE_ROW_BASE + len(dve_ops.OPS)
    assert row < 0x20
    shas = {}
    for ver in ("v3", "v4"):
        tmp = DveOpSpec(
            name=_SCAN_OP_NAME,
            opcode=row,
            uops=lower(spec, ver=ver),
            rd1_en=_has_src1(spec),
        )
        shas[ver] = tmp.sha(ver)
    op = dve_ops.DveOp(_SCAN_OP_NAME, spec, subdim=False, uops_sha=shas)
    dve_ops.OPS.append(op)
    dve_ops._SUB_OPCODE_FOR_NAME[_SCAN_OP_NAME] = row
    dve_ops.CUSTOM_DVE_SPECS[_SCAN_OP_NAME] = spec
    return op


def _build():
    import concourse.bass as bass
    import concourse.tile as tile
    from concourse import bacc, mybir, bass_isa

    scan_op = _register_scan_op()

    dt = mybir.dt
    nc = bacc.Bacc(
        "TRN2", target_bir_lowering=False, debug=False, num_devices=N_CORES
    )
    x_d = nc.dram_tensor("x", [B_LOC, N, D], dt.float32, kind="ExternalInput").ap()
    bias_d = nc.dram_tensor(
        "bias", [B_LOC, P, T], dt.float32, kind="ExternalInput"
    ).ap()
    q_d = nc.dram_tensor("q", [P, D], dt.float32, kind="ExternalInput").ap()
    out_d = nc.dram_tensor(
        "out", [B_LOC, 1, D], dt.float32, kind="ExternalOutput"
    ).ap()
    z_d = nc.dram_tensor("z", [B_LOC, P, 1], dt.float32, kind="ExternalOutput").ap()

    wdt = dt.bfloat16 if USE_BF16_PASS2 else dt.float32

    with tile.TileContext(nc) as tc:
        with (
            tc.tile_pool(name="singles", bufs=1) as singles,
            tc.tile_pool(name="xf32", bufs=10) as xf32,
            tc.tile_pool(name="xbf", bufs=2 * NCHUNK) as xbf,
            tc.tile_pool(name="small", bufs=2) as small,
            tc.tile_pool(name="psum", bufs=2, space="PSUM") as psum,
        ):
            qb = singles.tile([P, D], dt.float32)
            nc.scalar.dma_start(qb[:], q_d[:])
            q3 = qb.rearrange("p (u d) -> p u d", u=1).broadcast_to([P, T8, D])

            # segment-end accumulator: per chunk group, col 9c = 0 (set once),
            # cols 9c+1..9c+8 = running prefix at each 256-elem segment end.
            ends9 = singles.tile([P, NCHUNK * GK], dt.float32)
            nc.vector.memset(ends9[:], 0.0)

            for b in range(B_LOC):
                bias_t = small.tile([P, T], dt.float32)
                nc.scalar.dma_start(bias_t[:], bias_d[b])

                xrow = x_d[b].rearrange("(t8 p s) d -> p t8 s d", p=P, s=S)
                chunks = []
                bchunks = []
                for c in range(NCHUNK):
                    ch = xf32.tile([P, S, D], dt.float32)
                    dma_eng = nc.sync if c % 2 == 0 else nc.scalar
                    dma_eng.dma_start(ch[:], xrow[:, c])
                    chunks.append(ch)
                    if USE_BF16_PASS2:
                        cb = xbf.tile([P, S, D], dt.bfloat16)
                        nc.scalar.copy(cb[:], ch[:])
                        bchunks.append(cb)
                    else:
                        bchunks.append(ch)

                for c in range(NCHUNK):
                    o3 = (
                        ends9[:, c * GK + 1 : c * GK + 1 + S]
                        .rearrange("p (g u) -> p g u", u=1)
                        .broadcast_to([P, S, D])
                    )
                    nc.vector._custom_dve(
                        scan_op,
                        out=o3,
                        in0=chunks[c].rearrange("p s d -> p (s d)"),
                        in1=q3,
                    )

                # logits0[col] = ends[col] - ends[col-1] (zero at chunk starts)
                e9 = ends9.rearrange("p (g k) -> p g k", k=GK)
                logits0 = small.tile([P, T], dt.float32)
                nc.vector.tensor_tensor(
                    logits0.rearrange("p (g k) -> p g k", k=S),
                    e9[:, :, 1 : 1 + S],
                    e9[:, :, 0:S],
                    op=mybir.AluOpType.subtract,
                )
                logits = small.tile([P, T], dt.float32)
                nc.vector.tensor_tensor(
                    logits[:], logits0[:], bias_t[:], op=mybir.AluOpType.add
                )

                m = small.tile([P, 1], dt.float32)
                nc.vector.reduce_max(m[:], logits[:], axis=mybir.AxisListType.X)
                mall = small.tile([P, 1], dt.float32)
                nc.gpsimd.partition_all_reduce(
                    mall[:], m[:], channels=P, reduce_op=bass_isa.ReduceOp.max
                )
                negm = small.tile([P, 1], dt.float32)
                nc.vector.tensor_scalar_mul(negm[:], mall[:], -1.0)

                w = small.tile([P, T], wdt)
                zcol = small.tile([P, 1], dt.float32)
                nc.scalar.activation(
                    w[:],
                    logits[:],
                    mybir.ActivationFunctionType.Exp,
                    bias=negm[:],
                    accum_out=zcol[:],
                )
                nc.scalar.dma_start(z_d[b], zcol[:])

                # pass 2: out[1, d] += w[:, col].T @ x_tile; w column stationary
                # (M=1 -> LDWEIGHTS ~1 cycle); two PSUM banks ping-pong so the
                # systolic drain of matmul k overlaps the stream of k+1.
                acc = psum.tile([1, D], dt.float32)
                for c in range(NCHUNK):
                    cb = bchunks[c]
                    for s in range(S):
                        col = c * S + s
                        nc.tensor.matmul(
                            acc[:],
                            w[:, col : col + 1],
                            cb[:, s, :],
                            start=(col == 0),
                            stop=(col == T - 1),
                        )

                outsb = small.tile([1, D], dt.float32)
                nc.vector.tensor_copy(outsb[:], acc[:])
                nc.scalar.dma_start(out_d[b], outsb[:])

    nc.compile()
    return nc


def _prep_core_inputs(x, mask, q):
    """Host-side shard prep. Returns list of per-core input dicts."""
    qb = np.ascontiguousarray(np.broadcast_to(q[None, :], (P, D)), dtype=np.float32)
    # bias[b, p, col] for col = t8*8 + s, position n = t8*1024 + p*8 + s
    bias_all = np.where(mask, np.float32(0.0), np.float32(-1e30)).astype(np.float32)
    bias_all = bias_all.reshape(B, T8, P, S).transpose(0, 2, 1, 3).reshape(B, P, T)
    in_maps = []
    for i in range(N_CORES):
        sl = slice(i * B_LOC, (i + 1) * B_LOC)
        in_maps.append(
            {
                "x": np.ascontiguousarray(x[sl]),
                "bias": np.ascontiguousarray(bias_all[sl]),
                "q": qb,
            }
        )
    return in_maps


def kernel(x, mask, q, _trace=False):
    from concourse.bass_utils import run_bass_kernel_spmd

    x = np.asarray(x, dtype=np.float32)
    mask = np.asarray(mask)
    q = np.asarray(q, dtype=np.float32)
    assert x.shape == (B, N, D) and mask.shape == (B, N) and q.shape == (D,)

    if "nc" not in _cache:
        _cache["nc"] = _build()
    nc = _cache["nc"]

    in_maps = _prep_core_inputs(x, mask, q)
    res = run_bass_kernel_spmd(nc, in_maps, list(range(N_CORES)), trace=_trace)
    out = np.empty((B, D), dtype=np.float32)
    for i in range(N_CORES):
        o = res.results[i]["out"][:, 0, :]  # [B_LOC, D], unnormalized
        z = res.results[i]["z"][:, :, 0].astype(np.float64).sum(axis=1)  # [B_LOC]
        out[i * B_LOC : (i + 1) * B_LOC] = o / z[:, None]
    if _trace:
        return out, res
    return out


# revision 16
# speedup vs baseline: 1.0964x; 1.0964x over previous
"""AttentionPool Trainium2 kernel.

Computes, for x [B, N, D], mask [B, N], q [D]:
    logits = einsum('bnd,d->bn', x, q);  logits[~mask] = -inf
    w = softmax(logits, axis=-1)
    out = einsum('bn,bnd->bd', w, x)

Sharding: data-parallel over B across 8 NeuronCores (4 rows per core).

Position enumeration (per row): n = t8*1024 + p*8 + s, with p = SBUF
partition, s in [0,8), t8 in [0,8). Each partition reads 8 consecutive
positions = 8 KiB contiguous DRAM per (p, t8) -> one fat DMA descriptor.
A "tile" is (t8, s): 128 positions, one per partition; col = t8*8 + s.

Per-core device program (per batch row):
  - DMA the row into SBUF in 8 chunks (one per t8), f32.
  - ScalarE casts each chunk to bf16 (for pass 2).
  - Logits on DVE via a custom scan op (registered in-process; ships its own
    uop tables in the NEFF): one op per chunk computes the running prefix of
    x*q over 2048 elements; a stride-0 output AP keeps only each 256-element
    segment end -> 8 segment sums per op at ~1.09 cycles/element.
    Tile dot products = adjacent-difference of segment ends (one DVE
    subtract per row, with a zero column between chunk groups), + mask bias.
  - Row max: DVE free-dim reduce_max + GPSIMD partition_all_reduce(max).
  - w = exp(logits - max) on ScalarE (bf16 out), accum_out -> per-partition
    exp-sums (f32); Z summed on host.
  - Pass 2 on TensorE in bf16: out[1, d] += w[:, col].T @ x_tile[:, d]
    (stationary = the w column, M=1 -> LDWEIGHTS ~1 cycle), PSUM accumulate.
  - Host divides by Z.
"""

import numpy as np

B, N, D = 32, 8192, 256
N_CORES = 8
B_LOC = B // N_CORES  # 4
P = 128
S = 8               # consecutive positions per partition (8 KiB descriptors)
T8 = N // (P * S)   # 8 chunk groups per row
T = N // P          # 64 tiles (columns) per row
NCHUNK = T8         # one DMA chunk per t8 group
GK = 9              # ends layout: 1 zero col + 8 segment ends per chunk

USE_BF16_PASS2 = True

_cache = {}

_SCAN_OP_NAME = "ATTNPOOL_MUL_SCAN"


def _register_scan_op():
    """Register a custom DVE op computing scan(add, Src0*Src1) in-process.

    The stock TENSOR_TENSOR_REDUCE / TENSOR_TENSOR_SCAN opcodes crash this
    terminal's ucode; custom-DVE ops ship their own uop tables inside the
    NEFF, so they are self-contained.
    """
    from concourse import dve_ops
    from concourse.dve_spec import AluOp, Spec, Src0, Src1, scan, lower, _has_src1
    from concourse.dve_uop import DveOpSpec

    for op in dve_ops.OPS:
        if op.name == _SCAN_OP_NAME:
            return op
    spec = Spec(
        body=scan(AluOp.ADD, Src0 * Src1),
        reference=lambda in0, in1, c0, c1, c2: np.cumsum(
            in0.astype(np.float32) * in1, axis=1, dtype=np.float32
        ),
    )
    row = dve_ops._CUSTOM_DVE_ROW_BASE + len(dve_ops.OPS)
    assert row < 0x20
    shas = {}
    for ver in ("v3", "v4"):
        tmp = DveOpSpec(
            name=_SCAN_OP_NAME,
            opcode=row,
            uops=lower(spec, ver=ver),
            rd1_en=_has_src1(spec),
        )
        shas[ver] = tmp.sha(ver)
    op = dve_ops.DveOp(_SCAN_OP_NAME, spec, subdim=False, uops_sha=shas)
    dve_ops.OPS.append(op)
    dve_ops._SUB_OPCODE_FOR_NAME[_SCAN_OP_NAME] = row
    dve_ops.CUSTOM_DVE_SPECS[_SCAN_OP_NAME] = spec
    return op


def _build():
    import concourse.bass as bass
    import concourse.tile as tile
    from concourse import bacc, mybir, bass_isa

    scan_op = _register_scan_op()

    dt = mybir.dt
    nc = bacc.Bacc(
        "TRN2", target_bir_lowering=False, debug=False, num_devices=N_CORES
    )
    x_d = nc.dram_tensor("x", [B_LOC, N, D], dt.float32, kind="ExternalInput").ap()
    bias_d = nc.dram_tensor(
        "bias", [B_LOC, P, T], dt.float32, kind="ExternalInput"
    ).ap()
    q_d = nc.dram_tensor("q", [P, D], dt.float32, kind="ExternalInput").ap()
    out_d = nc.dram_tensor(
        "out", [B_LOC, 1, D], dt.float32, kind="ExternalOutput"
    ).ap()
    z_d = nc.dram_tensor("z", [B_LOC, P, 1], dt.float32, kind="ExternalOutput").ap()

    wdt = dt.bfloat16 if USE_BF16_PASS2 else dt.float32

    with tile.TileContext(nc) as tc:
        with (
            tc.tile_pool(name="singles", bufs=1) as singles,
            tc.tile_pool(name="xf32", bufs=10) as xf32,
            tc.tile_pool(name="xbf", bufs=2 * NCHUNK) as xbf,
            tc.tile_pool(name="small", bufs=2) as small,
            tc.tile_pool(name="psum", bufs=2, space="PSUM") as psum,
        ):
            qb = singles.tile([P, D], dt.float32)
            nc.scalar.dma_start(qb[:], q_d[:])
            q3 = qb.rearrange("p (u d) -> p u d", u=1).broadcast_to([P, T8, D])

            # segment-end accumulator: per chunk group, col 9c = 0 (set once),
            # cols 9c+1..9c+8 = running prefix at each 256-elem segment end.
            ends9 = singles.tile([P, NCHUNK * GK], dt.float32)
            nc.vector.memset(ends9[:], 0.0)

            for b in range(B_LOC):
                bias_t = small.tile([P, T], dt.float32)
                nc.scalar.dma_start(bias_t[:], bias_d[b])

                xrow = x_d[b].rearrange("(t8 p s) d -> p t8 s d", p=P, s=S)
                chunks = []
                bchunks = []
                for c in range(NCHUNK):
                    ch = xf32.tile([P, S, D], dt.float32)
                    nc.sync.dma_start(ch[:], xrow[:, c])
                    chunks.append(ch)
                    if USE_BF16_PASS2:
                        cb = xbf.tile([P, S, D], dt.bfloat16)
                        nc.scalar.copy(cb[:], ch[:])
                        bchunks.append(cb)
                    else:
                        bchunks.append(ch)

                for c in range(NCHUNK):
                    o3 = (
                        ends9[:, c * GK + 1 : c * GK + 1 + S]
                        .rearrange("p (g u) -> p g u", u=1)
                        .broadcast_to([P, S, D])
                    )
                    nc.vector._custom_dve(
                        scan_op,
                        out=o3,
                        in0=chunks[c].rearrange("p s d -> p (s d)"),
                        in1=q3,
                    )

                # logits0[col] = ends[col] - ends[col-1] (zero at chunk starts)
                e9 = ends9.rearrange("p (g k) -> p g k", k=GK)
                logits0 = small.tile([P, T], dt.float32)
                nc.vector.tensor_tensor(
                    logits0.rearrange("p (g k) -> p g k", k=S),
                    e9[:, :, 1 : 1 + S],
                    e9[:, :, 0:S],
                    op=mybir.AluOpType.subtract,
                )
                logits = small.tile([P, T], dt.float32)
                nc.vector.tensor_tensor(
                    logits[:], logits0[:], bias_t[:], op=mybir.AluOpType.add
                )

                m = small.tile([P, 1], dt.float32)
                nc.vector.reduce_max(m[:], logits[:], axis=mybir.AxisListType.X)
                mall = small.tile([P, 1], dt.float32)
                nc.gpsimd.partition_all_reduce(
                    mall[:], m[:], channels=P, reduce_op=bass_isa.ReduceOp.max
                )
                negm = small.tile([P, 1], dt.float32)
                nc.vector.tensor_scalar_mul(negm[:], mall[:], -1.0)

                w = small.tile([P, T], wdt)
                zcol = small.tile([P, 1], dt.float32)
                nc.scalar.activation(
                    w[:],
                    logits[:],
                    mybir.ActivationFunctionType.Exp,
                    bias=negm[:],
                    accum_out=zcol[:],
                )
                nc.scalar.dma_start(z_d[b], zcol[:])

                # pass 2: out[1, d] += w[:, col].T @ x_tile; w column stationary
                # (M=1 -> LDWEIGHTS ~1 cycle); two PSUM banks ping-pong so the
                # systolic drain of matmul k overlaps the stream of k+1.
                acc = psum.tile([1, D], dt.float32)
                for c in range(NCHUNK):
                    cb = bchunks[c]
                    for s in range(S):
                        col = c * S + s
                        nc.tensor.matmul(
                            acc[:],
                            w[:, col : col + 1],
                            cb[:, s, :],
                            start=(col == 0),
                            stop=(col == T - 1),
                        )

                outsb = small.tile([1, D], dt.float32)
                nc.vector.tensor_copy(outsb[:], acc[:])
                nc.scalar.dma_start(out_d[b], outsb[:])

    nc.compile()
    return nc


def _prep_core_inputs(x, mask, q):
    """Host-side shard prep. Returns list of per-core input dicts."""
    qb = np.ascontiguousarray(np.broadcast_to(q[None, :], (P, D)), dtype=np.float32)
    # bias[b, p, col] for col = t8*8 + s, position n = t8*1024 + p*8 + s
    bias_all = np.where(mask, np.float32(0.0), np.float32(-1e30)).astype(np.float32)
    bias_all = bias_all.reshape(B, T8, P, S).transpose(0, 2, 1, 3).reshape(B, P, T)
    in_maps = []
    for i in range(N_CORES):
        sl = slice(i * B_LOC, (i + 1) * B_LOC)
        in_maps.append(
            {
                "x": np.ascontiguousarray(x[sl]),
                "bias": np.ascontiguousarray(bias_all[sl]),
                "q": qb,
            }
        )
    return in_maps


def kernel(x, mask, q, _trace=False):
    from concourse.bass_utils import run_bass_kernel_spmd

    x = np.asarray(x, dtype=np.float32)
    mask = np.asarray(mask)
    q = np.asarray(q, dtype=np.float32)
    assert x.shape == (B, N, D) and mask.shape == (B, N) and q.shape == (D,)

    if "nc" not in _cache:
        _cache["nc"] = _build()
    nc = _cache["nc"]

    in_maps = _prep_core_inputs(x, mask, q)
    res = run_bass_kernel_spmd(nc, in_maps, list(range(N_CORES)), trace=_trace)
    out = np.empty((B, D), dtype=np.float32)
    for i in range(N_CORES):
        o = res.results[i]["out"][:, 0, :]  # [B_LOC, D], unnormalized
        z = res.results[i]["z"][:, :, 0].astype(np.float64).sum(axis=1)  # [B_LOC]
        out[i * B_LOC : (i + 1) * B_LOC] = o / z[:, None]
    if _trace:
        return out, res
    return out


# revision 20
# speedup vs baseline: 1.1098x; 1.0123x over previous
"""AttentionPool Trainium2 kernel.

Computes, for x [B, N, D], mask [B, N], q [D]:
    logits = einsum('bnd,d->bn', x, q);  logits[~mask] = -inf
    w = softmax(logits, axis=-1)
    out = einsum('bn,bnd->bd', w, x)

Sharding: data-parallel over B across 8 NeuronCores (4 rows per core).

Position enumeration (per row): n = t8*1024 + p*8 + s, with p = SBUF
partition, s in [0,8), t8 in [0,8). Each partition reads 8 consecutive
positions = 8 KiB contiguous DRAM per (p, t8) -> one fat DMA descriptor.
A "tile" is (t8, s): 128 positions, one per partition; col = t8*8 + s.

Per-core device program (per batch row):
  - DMA the row into SBUF in 8 chunks (one per t8), f32.
  - ScalarE casts each chunk to bf16 (for pass 2).
  - Logits on DVE via a custom scan op (registered in-process; ships its own
    uop tables in the NEFF): one op per chunk computes the running prefix of
    x*q over 2048 elements; a stride-0 output AP keeps only each 256-element
    segment end -> 8 segment sums per op at ~1.09 cycles/element.
    Tile dot products = adjacent-difference of segment ends (one DVE
    subtract per row, with a zero column between chunk groups), + mask bias.
  - Row max: DVE free-dim reduce_max + GPSIMD partition_all_reduce(max).
  - w = exp(logits - max) on ScalarE (bf16 out), accum_out -> per-partition
    exp-sums (f32); Z summed on host.
  - Pass 2 on TensorE in bf16: out[1, d] += w[:, col].T @ x_tile[:, d]
    (stationary = the w column, M=1 -> LDWEIGHTS ~1 cycle), PSUM accumulate.
  - Host divides by Z.
"""

import numpy as np

B, N, D = 32, 8192, 256
N_CORES = 8
B_LOC = B // N_CORES  # 4
P = 128
S = 8               # consecutive positions per partition (8 KiB descriptors)
T8 = N // (P * S)   # 8 chunk groups per row
T = N // P          # 64 tiles (columns) per row
NCHUNK = T8         # one DMA chunk per t8 group
GK = 9              # ends layout: 1 zero col + 8 segment ends per chunk

USE_BF16_PASS2 = True

_cache = {}

_SCAN_OP_NAME = "ATTNPOOL_MUL_SCAN"


def _register_scan_op():
    """Register a custom DVE op computing scan(add, Src0*Src1) in-process.

    The stock TENSOR_TENSOR_REDUCE / TENSOR_TENSOR_SCAN opcodes crash this
    terminal's ucode; custom-DVE ops ship their own uop tables inside the
    NEFF, so they are self-contained.
    """
    from concourse import dve_ops
    from concourse.dve_spec import AluOp, Spec, Src0, Src1, scan, lower, _has_src1
    from concourse.dve_uop import DveOpSpec

    for op in dve_ops.OPS:
        if op.name == _SCAN_OP_NAME:
            return op
    spec = Spec(
        body=scan(AluOp.ADD, Src0 * Src1),
        reference=lambda in0, in1, c0, c1, c2: np.cumsum(
            in0.astype(np.float32) * in1, axis=1, dtype=np.float32
        ),
    )
    row = dve_ops._CUSTOM_DVE_ROW_BASE + len(dve_ops.OPS)
    assert row < 0x20
    shas = {}
    for ver in ("v3", "v4"):
        tmp = DveOpSpec(
            name=_SCAN_OP_NAME,
            opcode=row,
            uops=lower(spec, ver=ver),
            rd1_en=_has_src1(spec),
        )
        shas[ver] = tmp.sha(ver)
    op = dve_ops.DveOp(_SCAN_OP_NAME, spec, subdim=False, uops_sha=shas)
    dve_ops.OPS.append(op)
    dve_ops._SUB_OPCODE_FOR_NAME[_SCAN_OP_NAME] = row
    dve_ops.CUSTOM_DVE_SPECS[_SCAN_OP_NAME] = spec
    return op


def _build():
    import concourse.bass as bass
    import concourse.tile as tile
    from concourse import bacc, mybir, bass_isa

    scan_op = _register_scan_op()

    dt = mybir.dt
    nc = bacc.Bacc(
        "TRN2", target_bir_lowering=False, debug=False, num_devices=N_CORES
    )
    x_d = nc.dram_tensor("x", [B_LOC, N, D], dt.float32, kind="ExternalInput").ap()
    bias_d = nc.dram_tensor(
        "bias", [B_LOC, P, T], dt.float32, kind="ExternalInput"
    ).ap()
    q_d = nc.dram_tensor("q", [P, D], dt.float32, kind="ExternalInput").ap()
    out_d = nc.dram_tensor(
        "out", [B_LOC, 1, D], dt.float32, kind="ExternalOutput"
    ).ap()
    z_d = nc.dram_tensor("z", [B_LOC, P, 1], dt.float32, kind="ExternalOutput").ap()
    c01_d = nc.dram_tensor("c01", [2, 1], dt.float32, kind="ExternalInput").ap()

    wdt = dt.bfloat16 if USE_BF16_PASS2 else dt.float32

    with tile.TileContext(nc) as tc:
        with (
            tc.tile_pool(name="singles", bufs=1) as singles,
            tc.tile_pool(name="xf32", bufs=10) as xf32,
            tc.tile_pool(name="xbf", bufs=2 * NCHUNK) as xbf,
            tc.tile_pool(name="small", bufs=2) as small,
            tc.tile_pool(name="psum", bufs=2, space="PSUM") as psum,
        ):
            qb = singles.tile([P, D], dt.float32)
            nc.scalar.dma_start(qb[:], q_d[:])
            sel1 = singles.tile([2, 1], dt.float32)
            nc.scalar.dma_start(sel1[:], c01_d[:])
            q3 = qb.rearrange("p (u d) -> p u d", u=1).broadcast_to([P, T8, D])

            # segment-end accumulator: per chunk group, col 9c = 0 (set once),
            # cols 9c+1..9c+8 = running prefix at each 256-elem segment end.
            ends9 = singles.tile([P, NCHUNK * GK], dt.float32)
            nc.vector.memset(ends9[:], 0.0)

            for b in range(B_LOC):
                bias_t = small.tile([P, T], dt.float32)
                nc.scalar.dma_start(bias_t[:], bias_d[b])

                xrow = x_d[b].rearrange("(t8 p s) d -> p t8 s d", p=P, s=S)
                chunks = []
                bchunks = []
                for c in range(NCHUNK):
                    ch = xf32.tile([P, S, D], dt.float32)
                    nc.sync.dma_start(ch[:], xrow[:, c])
                    chunks.append(ch)
                    if USE_BF16_PASS2:
                        cb = xbf.tile([P, S, D], dt.bfloat16)
                        nc.scalar.copy(cb[:], ch[:])
                        bchunks.append(cb)
                    else:
                        bchunks.append(ch)

                for c in range(NCHUNK):
                    o3 = (
                        ends9[:, c * GK + 1 : c * GK + 1 + S]
                        .rearrange("p (g u) -> p g u", u=1)
                        .broadcast_to([P, S, D])
                    )
                    nc.vector._custom_dve(
                        scan_op,
                        out=o3,
                        in0=chunks[c].rearrange("p s d -> p (s d)"),
                        in1=q3,
                    )

                # logits0[col] = ends[col] - ends[col-1] (zero at chunk starts)
                e9 = ends9.rearrange("p (g k) -> p g k", k=GK)
                logits0 = small.tile([P, T], dt.float32)
                nc.vector.tensor_tensor(
                    logits0.rearrange("p (g k) -> p g k", k=S),
                    e9[:, :, 1 : 1 + S],
                    e9[:, :, 0:S],
                    op=mybir.AluOpType.subtract,
                )
                logits = small.tile([P, T], dt.float32)
                nc.vector.tensor_tensor(
                    logits[:], logits0[:], bias_t[:], op=mybir.AluOpType.add
                )

                m = small.tile([P, 1], dt.float32)
                nc.vector.reduce_max(m[:], logits[:], axis=mybir.AxisListType.X)
                mall = small.tile([P, 1], dt.float32)
                nc.gpsimd.partition_all_reduce(
                    mall[:], m[:], channels=P, reduce_op=bass_isa.ReduceOp.max
                )
                negm = small.tile([P, 1], dt.float32)
                nc.vector.tensor_scalar_mul(negm[:], mall[:], -1.0)

                w = small.tile([P, T], wdt)
                zcol = small.tile([P, 1], dt.float32)
                nc.scalar.activation(
                    w[:],
                    logits[:],
                    mybir.ActivationFunctionType.Exp,
                    bias=negm[:],
                    accum_out=zcol[:],
                )
                nc.scalar.dma_start(z_d[b], zcol[:])

                # pass 2: out[1, d] += w[:, col].T @ x_tile; w column stationary
                # (M=1 -> LDWEIGHTS ~1 cycle); two PSUM banks ping-pong so the
                # systolic drain of matmul k overlaps the stream of k+1.
                # M=2: two adjacent tiles per matmul. lhsT = two w columns
                # [128, 2], rhs = their x tiles side by side [128, 512];
                # row result = acc[0, 0:256] + acc[1, 256:512] (off-diagonal
                # blocks are unused cross terms).
                acc = psum.tile([2, 2 * D], dt.float32)
                for c in range(NCHUNK):
                    cb = bchunks[c]
                    for s in range(0, S, 2):
                        col = c * S + s
                        nc.tensor.matmul(
                            acc[:],
                            w[:, col : col + 2],
                            cb[:, s : s + 2, :].rearrange("p s d -> p (s d)"),
                            start=(col == 0),
                            stop=(col == T - 2),
                        )

                halves = small.tile([2, 2 * D], dt.float32)
                nc.scalar.copy(halves[:], acc[:])
                # partition shift: psum2[0, :] = halves[1, D:2D] via lhsT=[0;1]
                psum2 = psum.tile([1, D], dt.float32, name="psum2", tag="p2")
                nc.tensor.matmul(psum2[:], sel1[:], halves[:, D : 2 * D])
                outsb = small.tile([1, D], dt.float32)
                nc.vector.tensor_tensor(
                    outsb[:], halves[0:1, 0:D], psum2[:],
                    op=mybir.AluOpType.add,
                )
                nc.scalar.dma_start(out_d[b], outsb[:])

    nc.compile()
    return nc


def _prep_core_inputs(x, mask, q):
    """Host-side shard prep. Returns list of per-core input dicts."""
    qb = np.ascontiguousarray(np.broadcast_to(q[None, :], (P, D)), dtype=np.float32)
    # bias[b, p, col] for col = t8*8 + s, position n = t8*1024 + p*8 + s
    bias_all = np.where(mask, np.float32(0.0), np.float32(-1e30)).astype(np.float32)
    bias_all = bias_all.reshape(B, T8, P, S).transpose(0, 2, 1, 3).reshape(B, P, T)
    in_maps = []
    for i in range(N_CORES):
        sl = slice(i * B_LOC, (i + 1) * B_LOC)
        in_maps.append(
            {
                "x": np.ascontiguousarray(x[sl]),
                "bias": np.ascontiguousarray(bias_all[sl]),
                "q": qb,
                "c01": np.array([[0.0], [1.0]], dtype=np.float32),
            }
        )
    return in_maps


def kernel(x, mask, q, _trace=False):
    from concourse.bass_utils import run_bass_kernel_spmd

    x = np.asarray(x, dtype=np.float32)
    mask = np.asarray(mask)
    q = np.asarray(q, dtype=np.float32)
    assert x.shape == (B, N, D) and mask.shape == (B, N) and q.shape == (D,)

    if "nc" not in _cache:
        _cache["nc"] = _build()
    nc = _cache["nc"]

    in_maps = _prep_core_inputs(x, mask, q)
    res = run_bass_kernel_spmd(nc, in_maps, list(range(N_CORES)), trace=_trace)
    out = np.empty((B, D), dtype=np.float32)
    for i in range(N_CORES):
        o = res.results[i]["out"][:, 0, :]  # [B_LOC, D], unnormalized
        z = res.results[i]["z"][:, :, 0].astype(np.float64).sum(axis=1)  # [B_LOC]
        out[i * B_LOC : (i + 1) * B_LOC] = o / z[:, None]
    if _trace:
        return out, res
    return out
